# revision 1
# baseline (speedup 1.0000x reference)
"""Trainium2 Bass kernel v2 for nn_CINComp.

out[b,o,d] = sum_{i,j} W[o,i*64+j]*feature[b,i,d]*base[b,j,d] + bias[o]

Data-parallel over batch B=1024 across 8 cores (BLOC=128 b/core). Per core:
  - all-bf16 datapath (accumulation fp32 in PSUM), output shipped bf16 and
    upcast on the host.
  - contraction dim ij=4096 -> 32 K-chunks of 128 = (2 i-rows x 64 j).
  - G resident transposed+duplicated gt2[(dup,j), bd] in bf16; each chunk's
    G-factor is a static partition slice.
  - F-broadcast per chunk via 16-tile-packed 32x32 selector matmuls on the
    PE (4 chunks per pack, ~240ns instead of 4 full 128-row matmuls).
  - fbc (PSUM fp32) -> P (SBUF bf16) via two tunable routes:
      ACT route:   ScalarE cast-copy PSUM->SBUF bf16, then DVE TT at 2x mode
      direct route: DVE TT fp32-PSUM x bf16 at 1x
      gpsimd route: ScalarE cast-copy, then GpSimd TT (frees DVE)
  - PE contracts W^T-chunk (bf16) @ P into PSUM acc, ScalarE adds bias
    during PSUM->SBUF bf16 drain, DMA out.
"""

import numpy as np
import ml_dtypes

import concourse.bass as bass
import concourse.mybir as mybir
import concourse.tile as tile
from concourse.bass import ts
from concourse.bass_utils import run_bass_kernel_spmd

B, HK, H0, D, O = 1024, 64, 64, 32, 128
NCORES = 8
BLOC = B // NCORES          # 128 batches per core
GROUPS = 8                  # bd groups per core
N = BLOC * D // GROUPS      # 512 = free dim per group
NCHUNK = 32                 # K chunks of 128 over ij=4096
NPACK = 8                   # chunk packs per group (4 chunks each)
F32 = mybir.dt.float32
BF16 = mybir.dt.bfloat16
BF = ml_dtypes.bfloat16

# Route assignment per pack (4 chunks): 'a' = ACT-copy + one DVE 2x mul,
# 'd' = two DVE direct 1x muls, 'g' = ACT-copy + GpSimd mul. Tuned on HW.
ROUTES = ['a', 'a', 'a', 'd', 'a', 'a', 'd', 'a',
          'a', 'a', 'd', 'a', 'a', 'd', 'a', 'd']

_CACHE = {}


def _sellib_const() -> np.ndarray:
    # sel128[p, 128*(c%16) + m] = 1 iff p % 32 == (2c + m//64) % 32:
    # K=32 x M=128 row-tile selector that broadcasts F rows {2c, 2c+1} of a
    # 32-row group across the two 64-partition halves in ONE matmul.
    sl = np.zeros((128, 16 * 128), np.float32)
    for cl in range(16):
        q = (2 * cl) % 32
        for m in range(128):
            k = q + m // 64
            for rg in range(4):
                sl[32 * rg + k, 128 * cl + m] = 1.0
    return sl.astype(BF)


def _strip_self_waits(nc: bass.Bass) -> None:
    """Transitively-minimal semaphore waits (see baseline docstring)."""
    UPD = ("sem-inc", "sem-add-imm")
    insts = [i for bb in nc.m.functions[0].blocks for i in bb.instructions]

    bad_sems = set()
    for i in insts:
        si = getattr(i, "sync_info", None)
        if si is None:
            continue
        for u in si.on_update:
            if u.sync_type != "semaphore" or u.update_mode not in UPD:
                bad_sems.add(u.id)

    def fifo_of(i):
        si = i.sync_info
        eng = str(getattr(i, "engine", None))
        if type(i).__name__ == "InstDMACopy" and si is not None:
            for u in si.on_update:
                if u.sync_type == "semaphore" and u.update_mode in UPD:
                    return ("q", u.id)
        return ("e", eng)

    cum: dict = {}
    event: dict = {}
    fifo_pred: dict = {}
    last_in_fifo: dict = {}
    metas = []
    for idx, i in enumerate(insts):
        si = getattr(i, "sync_info", None)
        f = fifo_of(i)
        fifo_pred[idx] = last_in_fifo.get(f)
        last_in_fifo[f] = idx
        ups = []
        if si is not None:
            for u in si.on_update:
                if u.sync_type == "semaphore" and u.update_mode in UPD:
                    cum[u.id] = cum.get(u.id, 0) + u.update_value
                    event[(u.id, cum[u.id])] = idx
                    ups.append((u.id, cum[u.id]))
        metas.append((si, ups))

    def resolve(sem, k):
        v = k
        while (sem, v) not in event:
            v += 1
            if v > cum.get(sem, 0):
                return None
        return event[(sem, v)]

    cvc: list = [None] * len(insts)

    def get_cvc(idx):
        if cvc[idx] is not None:
            return cvc[idx]
        stack = [idx]
        while stack:
            j = stack[-1]
            if cvc[j] is not None:
                stack.pop()
                continue
            si, ups = metas[j]
            deps = []
            p = fifo_pred[j]
            if p is not None:
                deps.append(p)
            if si is not None:
                for w in si.on_wait:
                    if (
                        w.sync_type == "semaphore"
                        and w.wait_mode == "sem-ge-imm"
                        and w.id not in bad_sems
                    ):
                        e = resolve(w.id, w.wait_value)
                        if e is not None and e != j:
                            deps.append(e)
            pending = [d for d in deps if cvc[d] is None]
            if pending:
                stack.extend(pending)
                continue
            stack.pop()
            vc: dict = {}
            for d in deps:
                for s, v in cvc[d].items():
                    if vc.get(s, 0) < v:
                        vc[s] = v
            if si is not None:
                for w in si.on_wait:
                    if (
                        w.sync_type == "semaphore"
                        and w.wait_mode == "sem-ge-imm"
                        and w.id not in bad_sems
                    ):
                        if vc.get(w.id, 0) < w.wait_value:
                            vc[w.id] = w.wait_value
            for s, v in ups:
                if vc.get(s, 0) < v:
                    vc[s] = v
            cvc[j] = vc
        return cvc[idx]

    for idx, i in enumerate(insts):
        si, _ups = metas[idx]
        if si is None or not si.on_wait:
            continue
        base: dict = {}
        p = fifo_pred[idx]
        if p is not None:
            base = dict(get_cvc(p))
        sem_waits = [
            w
            for w in si.on_wait
            if w.sync_type == "semaphore"
            and w.wait_mode == "sem-ge-imm"
            and w.id not in bad_sems
        ]
        other = [w for w in si.on_wait if w not in sem_waits]

        def strength(w):
            e = resolve(w.id, w.wait_value)
            return len(get_cvc(e)) if e is not None else 0

        sem_waits.sort(key=strength, reverse=True)

        def wait_cvc(w):
            e = resolve(w.id, w.wait_value)
            vc = dict(get_cvc(e)) if e is not None else {}
            if vc.get(w.id, 0) < w.wait_value:
                vc[w.id] = w.wait_value
            return vc

        kept = sem_waits[:]
        changed = True
        while changed:
            changed = False
            for w in kept:
                cover = dict(base)
                for w2 in kept:
                    if w2 is w:
                        continue
                    for s, v in wait_cvc(w2).items():
                        if cover.get(s, 0) < v:
                            cover[s] = v
                if cover.get(w.id, 0) >= w.wait_value:
                    kept.remove(w)
                    changed = True
                    break
        if len(kept) + len(other) != len(si.on_wait):
            si.on_wait = other + kept


def _build_nc(strip: bool = True) -> bass.Bass:
    nc = bass.Bass()
    wt = nc.dram_tensor("wt", [128, NCHUNK * 128], BF16, kind="ExternalInput")
    gt2 = nc.dram_tensor("gt2", [128, BLOC * D], BF16, kind="ExternalInput")
    ft4 = nc.dram_tensor("ft4", [128, BLOC * D], BF16, kind="ExternalInput")
    sel = nc.dram_tensor("sel", [128, 2048], BF16, kind="ExternalInput")
    bias = nc.dram_tensor("bias", [128, 1], F32, kind="ExternalInput")
    out = nc.dram_tensor("out", [128, BLOC * D], BF16, kind="ExternalOutput")

    # pack p covers chunks {p, p+8, p+16, p+24} at row-groups {0, 2, 1, 3}
    # (ft4 holds F rows 0..63 duplicated at partitions 64..127)
    PACK_CHUNKS = [(p, p + 8, p + 16, p + 24) for p in range(NPACK)]
    PACK_RGS = (0, 2, 1, 3)

    with tile.TileContext(nc) as tc:
        with (
            tc.tile_pool(name="res", bufs=1) as res,
            tc.tile_pool(name="pp", bufs=4) as ppool,
            tc.tile_pool(name="fbb", bufs=6) as fbpool,
            tc.tile_pool(name="osb", bufs=6) as opool,
            tc.tile_pool(name="fbc", bufs=3, space="PSUM") as fpool,
            tc.tile_pool(name="acc", bufs=2, space="PSUM") as apool,
        ):
            use_g = 'g' in ROUTES
            gt2_sb = res.tile([128, BLOC * D], BF16)
            gt2g_sb = None
            if use_g:
                gt2g_sb = res.tile([128, BLOC * D], BF16, name="gt2g_sb")
            ft4_sb = res.tile([128, BLOC * D], BF16)
            wt_sb = res.tile([128, NCHUNK * 128], BF16)
            sel_sb = res.tile([128, 2048], BF16)
            bias_sb = res.tile([128, 1], F32)

            # interleaved quarter loads so compute can start early; touch each
            # piece on its consumer engine (one-wait rule)
            Q = BLOC * D // 4
            nc.sync.dma_start(out=bias_sb[:], in_=bias[:])
            nc.sync.dma_start(out=sel_sb[:], in_=sel[:])
            nc.vector.tensor_copy(sel_sb[0:1, 0:1], sel_sb[0:1, 0:1])
            nc.vector.tensor_copy(bias_sb[0:1, 0:1], bias_sb[0:1, 0:1])
            for q in range(4):
                nc.sync.dma_start(out=ft4_sb[:, ts(q, Q)], in_=ft4[:, ts(q, Q)])
                nc.sync.dma_start(out=gt2_sb[:, ts(q, Q)], in_=gt2[:, ts(q, Q)])
                if use_g:
                    nc.sync.dma_start(out=gt2g_sb[:, ts(q, Q)],
                                      in_=gt2[:, ts(q, Q)])
                nc.sync.dma_start(out=wt_sb[:, ts(q, Q)], in_=wt[:, ts(q, Q)])
                for t in (ft4_sb, gt2_sb, wt_sb):
                    nc.vector.tensor_copy(t[0:1, q * Q:q * Q + 1],
                                          t[0:1, q * Q:q * Q + 1])
                if use_g:
                    # gpsimd reads its own gt2 copy; in-place touch on gpsimd
                    nc.gpsimd.tensor_copy(gt2g_sb[0:1, q * Q:q * Q + 1],
                                          gt2g_sb[0:1, q * Q:q * Q + 1])

            def bcast_pack(g, pk):
                """4 row-tile matmuls (K=32, M=128): broadcast the 4 chunks of
                pack pk for group g into two PSUM tiles [128, 2, N] fp32."""
                chunks = PACK_CHUNKS[pk]
                fbh0 = fpool.tile([128, 2, N], F32, tag="fbc")
                fbh1 = fpool.tile([128, 2, N], F32, tag="fbc")
                tiles = [fbh0, fbh1]
                for idx in range(4):
                    c, rg = chunks[idx], PACK_RGS[idx]
                    fb, slot = tiles[idx // 2], idx % 2
                    nc.tensor.matmul(
                        fb[:, slot, :],
                        sel_sb[32 * rg:32 * rg + 32,
                               128 * (c % 16):128 * (c % 16) + 128],
                        ft4_sb[32 * rg:32 * rg + 32, ts(g, N)],
                        start=True, stop=True,
                        tile_position=(32 * rg, 0))
                return tiles

            def mul_half(g, fb, route):
                """fbc tile [128,2,N] fp32 PSUM -> p tile [128,2,N] bf16."""
                gv2 = gt2_sb[:, ts(g, N)][:, None, :].to_broadcast((128, 2, N))
                ptag = "pg" if route == 'g' else "p"
                p2 = ppool.tile([128, 2, N], BF16, tag=ptag, bufs=12)
                if route in ('a', 'g'):
                    tag = "fbb" if route == 'a' else "fbbg"
                    fbb = fbpool.tile([128, 2, N], BF16, tag=tag, bufs=6)
                    nc.scalar.activation(fbb[:], fb[:],
                                         mybir.ActivationFunctionType.Copy)
                    if route == 'a':
                        nc.vector.tensor_mul(p2[:], fbb[:], gv2)
                    else:
                        gv2g = (gt2g_sb[:, ts(g, N)][:, None, :]
                                .to_broadcast((128, 2, N)))
                        nc.gpsimd.memset(p2[0:1, 0:1, 0:2].bitcast(F32), 0.0)
                        nc.gpsimd.tensor_mul(p2[:], fbb[:], gv2g)
                else:
                    nc.vector.tensor_mul(p2[:], fb[:], gv2)
                return p2

            def contract(acc, pps, chunks4, s0, s1):
                pa, pb = pps
                for s, c in enumerate(chunks4):
                    p2 = pa if s < 2 else pb
                    nc.tensor.matmul(acc[:], wt_sb[:, ts(c, 128)],
                                     p2[:, s % 2, :],
                                     start=s0 and s == 0,
                                     stop=s1 and s == 3)

            def drain(g, acc):
                osb = opool.tile([128, N], BF16, tag="osb")
                nc.scalar.activation(osb[:], acc[:],
                                     mybir.ActivationFunctionType.Identity,
                                     bias=bias_sb[:, 0:1])
                nc.sync.dma_start(out=out[:, ts(g, N)], in_=osb[:])
                nc.vector.tensor_copy(osb[0:1, 0:1], osb[0:1, 0:1])

            # software pipeline across (group, pack); contracts lag the
            # broadcast+mul by 2 packs so every mul has ~2.4us of slack
            pend = []  # (acc, p2-halves, chunks, is_first, is_last, g)
            LAG = 2
            for g in range(GROUPS):
                acc = apool.tile([128, N], F32, tag="acc")
                for pk in range(NPACK):
                    fts = bcast_pack(g, pk)
                    r0 = ROUTES[(2 * pk) % len(ROUTES)]
                    r1 = ROUTES[(2 * pk + 1) % len(ROUTES)]
                    pa = mul_half(g, fts[0], r0)
                    pb = mul_half(g, fts[1], r1)
                    pend.append((acc, (pa, pb), PACK_CHUNKS[pk], pk == 0,
                                 pk == NPACK - 1, g))
                    if len(pend) > LAG:
                        pacc, pps, pchunks, pfirst, plast, pg = pend.pop(0)
                        contract(pacc, pps, pchunks, pfirst, plast)
                        if plast:
                            drain(pg, pacc)
            for pacc, pps, pchunks, pfirst, plast, pg in pend:
                contract(pacc, pps, pchunks, pfirst, plast)
                if plast:
                    drain(pg, pacc)
    if strip:
        _strip_self_waits(nc)
    return nc


def _get_nc() -> bass.Bass:
    if "nc" not in _CACHE:
        _CACHE["nc"] = _build_nc()
    return _CACHE["nc"]


def _prep_core_inputs(feature, base, W, b, ci):
    bsl = slice(ci * BLOC, (ci + 1) * BLOC)
    F = np.asarray(feature[bsl], np.float32)  # (128, 64, 32)
    G = np.asarray(base[bsl], np.float32)     # (128, 64, 32)

    Gt = np.transpose(G, (1, 0, 2))                      # (j, b, d)
    gt2 = np.concatenate([Gt, Gt], 0).reshape(128, BLOC * D)

    Ft = np.transpose(F, (1, 0, 2))                      # (i, b, d)
    ft4 = np.concatenate([Ft, Ft], 0).reshape(128, BLOC * D)

    # wt[p=(delta,j), 128c + o] = W[o, (2c+delta)*64 + j]
    Wr = np.asarray(W, np.float32).reshape(O, NCHUNK, 2, H0)  # o, c, delta, j
    wt = np.transpose(Wr, (2, 3, 1, 0)).reshape(128, NCHUNK * 128)

    return {
        "wt": np.ascontiguousarray(wt).astype(BF),
        "gt2": np.ascontiguousarray(gt2).astype(BF),
        "ft4": np.ascontiguousarray(ft4).astype(BF),
        "bias": np.ascontiguousarray(b, np.float32).reshape(128, 1),
        "sel": _sellib_const(),
    }


def run(feature, base, W, b, **spmd_kwargs):
    nc = _get_nc()
    in_maps = [_prep_core_inputs(feature, base, W, b, ci) for ci in range(NCORES)]
    res = run_bass_kernel_spmd(nc, in_maps, list(range(NCORES)), **spmd_kwargs)
    outs = []
    for ci in range(NCORES):
        o = np.asarray(res.results[ci]["out"], dtype=np.float32)
        o = o.reshape(O, BLOC, D)
        outs.append(np.transpose(o, (1, 0, 2)))
    full = np.concatenate(outs, 0)
    return full, res


def kernel(feature, base, W, b):
    full, _ = run(feature, base, W, b)
    return full



# revision 2
# speedup vs baseline: 1.3588x; 1.3588x over previous
"""Trainium2 Bass kernel v3 for nn_CINComp.

out[b,o,d] = sum_{i,j} W[o,i*64+j]*feature[b,i,d]*base[b,j,d] + bias[o]

Data-parallel over batch B=1024 across 8 cores (BLOC=128 b/core).

v3 design ("dup-layout"): chunk the ij=4096 contraction into 32 chunks of
128 = (8 i's x 16 j's). Host-side duplicates BOTH factors to the chunk
partition layout p=(a,b): ftd dups each f-row x16 (8 MB/core), gtd dups
each g-row x8 (4 MB/core). The P=f*g product is then ONE unit-stride
bf16 SBUF DVE tensor_mul per (bd-quarter, i-block) at 2x mode -- no PE
broadcast matmuls, no PSUM intermediate, no ScalarE casts (vs v2 which
spent ~100us PE + ~90us ACT on that). PE does only the 256 main
contraction matmuls (~55us); DVE (~70us) is the pacer.

  - bd=4096 per core split into 4 quarters of 1024; PSUM = 8 banks = 4
    quarters x 2 accumulators [128,512], drained (bias add + bf16 cast)
    on ScalarE per quarter.
  - ~15 MB/core HBM traffic streamed in h-major slices so compute
    starts after ~1.3 MB.
  - dummy matmuls on scratch tiles during the DMA ramp keep/get the PE
    HAM clock warm (2.4 GHz) before the first real matmul.
"""

import numpy as np
import ml_dtypes

import concourse.bass as bass
import concourse.mybir as mybir
import concourse.tile as tile
from concourse.bass_utils import run_bass_kernel_spmd

B, HK, H0, D, O = 1024, 64, 64, 32, 128
NCORES = 8
BLOC = B // NCORES          # 128 batches per core
BD = BLOC * D               # 4096 bd points per core
NQ = 4                      # bd quarters
QL = BD // NQ               # 1024
IB = 8                      # i-blocks (8 i's each) -> ftd dup x16
JB = 4                      # j-blocks (16 j's each) -> gtd dup x8
NCHUNK = IB * JB            # 32 chunks of 128 = (a,b) partitions
NWARM = 10                  # PE warm-up matmuls during DMA ramp
F32 = mybir.dt.float32
BF16 = mybir.dt.bfloat16
BF = ml_dtypes.bfloat16

_CACHE = {}


def _strip_self_waits(nc: bass.Bass) -> None:
    """Transitively-minimal semaphore waits (see v2 docstring)."""
    UPD = ("sem-inc", "sem-add-imm")
    insts = [i for bb in nc.m.functions[0].blocks for i in bb.instructions]

    bad_sems = set()
    for i in insts:
        si = getattr(i, "sync_info", None)
        if si is None:
            continue
        for u in si.on_update:
            if u.sync_type != "semaphore" or u.update_mode not in UPD:
                bad_sems.add(u.id)

    def fifo_of(i):
        si = i.sync_info
        eng = str(getattr(i, "engine", None))
        if type(i).__name__ == "InstDMACopy" and si is not None:
            for u in si.on_update:
                if u.sync_type == "semaphore" and u.update_mode in UPD:
                    return ("q", u.id)
        return ("e", eng)

    cum: dict = {}
    event: dict = {}
    fifo_pred: dict = {}
    last_in_fifo: dict = {}
    metas = []
    for idx, i in enumerate(insts):
        si = getattr(i, "sync_info", None)
        f = fifo_of(i)
        fifo_pred[idx] = last_in_fifo.get(f)
        last_in_fifo[f] = idx
        ups = []
        if si is not None:
            for u in si.on_update:
                if u.sync_type == "semaphore" and u.update_mode in UPD:
                    cum[u.id] = cum.get(u.id, 0) + u.update_value
                    event[(u.id, cum[u.id])] = idx
                    ups.append((u.id, cum[u.id]))
        metas.append((si, ups))

    def resolve(sem, k):
        v = k
        while (sem, v) not in event:
            v += 1
            if v > cum.get(sem, 0):
                return None
        return event[(sem, v)]

    cvc: list = [None] * len(insts)

    def get_cvc(idx):
        if cvc[idx] is not None:
            return cvc[idx]
        stack = [idx]
        while stack:
            j = stack[-1]
            if cvc[j] is not None:
                stack.pop()
                continue
            si, ups = metas[j]
            deps = []
            p = fifo_pred[j]
            if p is not None:
                deps.append(p)
            if si is not None:
                for w in si.on_wait:
                    if (
                        w.sync_type == "semaphore"
                        and w.wait_mode == "sem-ge-imm"
                        and w.id not in bad_sems
                    ):
                        e = resolve(w.id, w.wait_value)
                        if e is not None and e != j:
                            deps.append(e)
            pending = [d for d in deps if cvc[d] is None]
            if pending:
                stack.extend(pending)
                continue
            stack.pop()
            vc: dict = {}
            for d in deps:
                for s, v in cvc[d].items():
                    if vc.get(s, 0) < v:
                        vc[s] = v
            if si is not None:
                for w in si.on_wait:
                    if (
                        w.sync_type == "semaphore"
                        and w.wait_mode == "sem-ge-imm"
                        and w.id not in bad_sems
                    ):
                        if vc.get(w.id, 0) < w.wait_value:
                            vc[w.id] = w.wait_value
            for s, v in ups:
                if vc.get(s, 0) < v:
                    vc[s] = v
            cvc[j] = vc
        return cvc[idx]

    for idx, i in enumerate(insts):
        si, _ups = metas[idx]
        if si is None or not si.on_wait:
            continue
        base: dict = {}
        p = fifo_pred[idx]
        if p is not None:
            base = dict(get_cvc(p))
        sem_waits = [
            w
            for w in si.on_wait
            if w.sync_type == "semaphore"
            and w.wait_mode == "sem-ge-imm"
            and w.id not in bad_sems
        ]
        other = [w for w in si.on_wait if w not in sem_waits]

        def strength(w):
            e = resolve(w.id, w.wait_value)
            return len(get_cvc(e)) if e is not None else 0

        sem_waits.sort(key=strength, reverse=True)

        def wait_cvc(w):
            e = resolve(w.id, w.wait_value)
            vc = dict(get_cvc(e)) if e is not None else {}
            if vc.get(w.id, 0) < w.wait_value:
                vc[w.id] = w.wait_value
            return vc

        kept = sem_waits[:]
        changed = True
        while changed:
            changed = False
            for w in kept:
                cover = dict(base)
                for w2 in kept:
                    if w2 is w:
                        continue
                    for s, v in wait_cvc(w2).items():
                        if cover.get(s, 0) < v:
                            cover[s] = v
                if cover.get(w.id, 0) >= w.wait_value:
                    kept.remove(w)
                    changed = True
                    break
        if len(kept) + len(other) != len(si.on_wait):
            si.on_wait = other + kept


def _build_nc(strip: bool = True) -> bass.Bass:
    nc = bass.Bass()
    ftd = nc.dram_tensor("ftd", [128, NQ, IB, QL], BF16, kind="ExternalInput")
    gtd = nc.dram_tensor("gtd", [128, NQ, JB, QL], BF16, kind="ExternalInput")
    wt = nc.dram_tensor("wt", [128, NCHUNK, 128], BF16, kind="ExternalInput")
    bias = nc.dram_tensor("bias", [128, 1], F32, kind="ExternalInput")
    out = nc.dram_tensor("out", [128, BD], BF16, kind="ExternalOutput")

    with tile.TileContext(nc) as tc:
        with (
            tc.tile_pool(name="res", bufs=1) as res,
            tc.tile_pool(name="pp", bufs=4) as ppool,
            tc.tile_pool(name="osb", bufs=4) as opool,
            tc.tile_pool(name="acc", bufs=8, space="PSUM") as apool,
        ):
            ftd_sb = res.tile([128, NQ * IB, QL], BF16)
            gtd_sb = res.tile([128, NQ * JB, QL], BF16)
            wt_sb = res.tile([128, NCHUNK, 128], BF16)
            bias_sb = res.tile([128, 1], F32)
            wl_sb = res.tile([128, 128], BF16, name="wl")
            wr_sb = res.tile([128, 512], BF16, name="wr")

            # PE warm-up: garbage matmuls on scratch tiles into the first
            # acc rotation slot keep the HAM clock busy during the DMA
            # ramp; bank is re-claimed by the last real accumulator long
            # after.  memset so CoreSim never sees uninitialized reads.
            nc.vector.memset(wl_sb[:], 0.0)
            nc.vector.memset(wr_sb[:], 0.0)
            warm = apool.tile([128, 512], F32, tag="acc")
            for k in range(NWARM):
                nc.tensor.matmul(warm[:], wl_sb[:], wr_sb[:],
                                 start=(k == 0), stop=(k == NWARM - 1))

            # streamed loads, h-major; first-use slices first.  Touch each
            # piece on its consumer engine (one-wait rule).
            nc.sync.dma_start(out=bias_sb[:], in_=bias[:])
            nc.sync.dma_start(out=gtd_sb[:, 0:JB, :], in_=gtd[:, 0, :, :])
            nc.vector.tensor_copy(gtd_sb[0:1, 0:1, 0:1], gtd_sb[0:1, 0:1, 0:1])
            for ib in range(IB):
                nc.sync.dma_start(out=ftd_sb[:, ib, :], in_=ftd[:, 0, ib, :])
                nc.vector.tensor_copy(ftd_sb[0:1, ib, 0:1],
                                      ftd_sb[0:1, ib, 0:1])
            nc.sync.dma_start(out=wt_sb[:], in_=wt[:])
            nc.vector.tensor_copy(wt_sb[0:1, 0:1, 0:1], wt_sb[0:1, 0:1, 0:1])
            for h in range(1, NQ):
                nc.sync.dma_start(out=gtd_sb[:, h * JB:(h + 1) * JB, :],
                                  in_=gtd[:, h, :, :])
                nc.vector.tensor_copy(gtd_sb[0:1, h * JB, 0:1],
                                      gtd_sb[0:1, h * JB, 0:1])
                nc.sync.dma_start(out=ftd_sb[:, h * IB:(h + 1) * IB, :],
                                  in_=ftd[:, h, :, :])
                nc.vector.tensor_copy(ftd_sb[0:1, h * IB, 0:1],
                                      ftd_sb[0:1, h * IB, 0:1])

            for h in range(NQ):
                acc0 = apool.tile([128, 512], F32, tag="acc")
                acc1 = apool.tile([128, 512], F32, tag="acc")
                gsl = gtd_sb[:, h * JB:(h + 1) * JB, :]
                for ib in range(IB):
                    p2 = ppool.tile([128, JB, QL], BF16, tag="p")
                    fap = (ftd_sb[:, h * IB + ib, :][:, None, :]
                           .to_broadcast((128, JB, QL)))
                    nc.vector.tensor_mul(p2[:], gsl, fap)
                    for jb in range(JB):
                        c = JB * ib + jb
                        st = (ib == 0 and jb == 0)
                        sp = (ib == IB - 1 and jb == JB - 1)
                        nc.tensor.matmul(acc0[:], wt_sb[:, c, :],
                                         p2[:, jb, 0:512], start=st, stop=sp)
                        nc.tensor.matmul(acc1[:], wt_sb[:, c, :],
                                         p2[:, jb, 512:QL], start=st, stop=sp)
                for q, acc in ((0, acc0), (1, acc1)):
                    osb = opool.tile([128, 512], BF16, tag="osb")
                    nc.scalar.activation(osb[:], acc[:],
                                         mybir.ActivationFunctionType.Identity,
                                         bias=bias_sb[:, 0:1])
                    nc.sync.dma_start(
                        out=out[:, h * QL + q * 512: h * QL + (q + 1) * 512],
                        in_=osb[:])
                    nc.vector.tensor_copy(osb[0:1, 0:1], osb[0:1, 0:1])
    if strip:
        _strip_self_waits(nc)
    return nc


def _get_nc() -> bass.Bass:
    if "nc" not in _CACHE:
        _CACHE["nc"] = _build_nc()
    return _CACHE["nc"]


def _prep_core_inputs(feature, base, W, b, ci):
    bsl = slice(ci * BLOC, (ci + 1) * BLOC)
    F = np.asarray(feature[bsl], np.float32)  # (128, 64, 32)
    G = np.asarray(base[bsl], np.float32)     # (128, 64, 32)

    fT = np.transpose(F, (1, 0, 2)).reshape(HK, BD)   # (i, bd)
    gT = np.transpose(G, (1, 0, 2)).reshape(H0, BD)   # (j, bd)

    # ftd[p=(16a+b), h, ib, l] = fT[8*ib + a, h*QL + l]   (dup x16 over b)
    t = fT.reshape(IB, 8, NQ, QL).transpose(1, 2, 0, 3)      # (a, h, ib, l)
    ftd = np.broadcast_to(t[:, None], (8, 16, NQ, IB, QL))
    ftd = ftd.reshape(128, NQ, IB, QL)

    # gtd[p=(16a+b), h, jb, l] = gT[16*jb + b, h*QL + l]   (dup x8 over a)
    t = gT.reshape(JB, 16, NQ, QL).transpose(1, 2, 0, 3)     # (b, h, jb, l)
    gtd = np.broadcast_to(t[None], (8, 16, NQ, JB, QL))
    gtd = gtd.reshape(128, NQ, JB, QL)

    # wt[p=(16a+b), c=(4*ib+jb), o] = W[o, (8*ib+a)*64 + 16*jb + b]
    wt = np.asarray(W, np.float32).reshape(O, IB, 8, JB, 16)
    wt = wt.transpose(2, 4, 1, 3, 0).reshape(128, NCHUNK, O)

    return {
        "ftd": np.ascontiguousarray(ftd).astype(BF),
        "gtd": np.ascontiguousarray(gtd).astype(BF),
        "wt": np.ascontiguousarray(wt).astype(BF),
        "bias": np.ascontiguousarray(b, np.float32).reshape(128, 1),
    }


def run(feature, base, W, b, **spmd_kwargs):
    nc = _get_nc()
    in_maps = [_prep_core_inputs(feature, base, W, b, ci) for ci in range(NCORES)]
    res = run_bass_kernel_spmd(nc, in_maps, list(range(NCORES)), **spmd_kwargs)
    outs = []
    for ci in range(NCORES):
        o = np.asarray(res.results[ci]["out"], dtype=np.float32)
        o = o.reshape(O, BLOC, D)
        outs.append(np.transpose(o, (1, 0, 2)))
    full = np.concatenate(outs, 0)
    return full, res


def kernel(feature, base, W, b):
    full, _ = run(feature, base, W, b)
    return full


# revision 3
# speedup vs baseline: 1.4222x; 1.0466x over previous
"""Trainium2 Bass kernel v3 for nn_CINComp.

out[b,o,d] = sum_{i,j} W[o,i*64+j]*feature[b,i,d]*base[b,j,d] + bias[o]

Data-parallel over batch B=1024 across 8 cores (BLOC=128 b/core).

v3 design ("dup-layout"): chunk the ij=4096 contraction into 32 chunks of
128 = (8 i's x 16 j's). Host-side duplicates BOTH factors to the chunk
partition layout p=(a,b): ftd dups each f-row x16 (8 MB/core), gtd dups
each g-row x8 (4 MB/core). The P=f*g product is then ONE unit-stride
bf16 SBUF DVE tensor_mul per (bd-quarter, i-block) at 2x mode -- no PE
broadcast matmuls, no PSUM intermediate, no ScalarE casts (vs v2 which
spent ~100us PE + ~90us ACT on that). PE does only the 256 main
contraction matmuls (~55us); DVE (~70us) is the pacer.

  - bd=4096 per core split into 4 quarters of 1024; PSUM = 8 banks = 4
    quarters x 2 accumulators [128,512], drained (bias add + bf16 cast)
    on ScalarE per quarter.
  - ~15 MB/core HBM traffic streamed in h-major slices so compute
    starts after ~1.3 MB.
  - dummy matmuls on scratch tiles during the DMA ramp keep/get the PE
    HAM clock warm (2.4 GHz) before the first real matmul.
"""

import numpy as np
import ml_dtypes

import concourse.bass as bass
import concourse.mybir as mybir
import concourse.tile as tile
from concourse.bass_utils import run_bass_kernel_spmd

B, HK, H0, D, O = 1024, 64, 64, 32, 128
NCORES = 8
BLOC = B // NCORES          # 128 batches per core
BD = BLOC * D               # 4096 bd points per core
NQ = 4                      # bd quarters
QL = BD // NQ               # 1024
IB = 8                      # i-blocks (8 i's each) -> ftd dup x16
JB = 4                      # j-blocks (16 j's each) -> gtd dup x8
NCHUNK = IB * JB            # 32 chunks of 128 = (a,b) partitions
NWARM = 10                  # PE warm-up matmuls during DMA ramp
F32 = mybir.dt.float32
BF16 = mybir.dt.bfloat16
BF = ml_dtypes.bfloat16

_CACHE = {}


def _strip_self_waits(nc: bass.Bass) -> None:
    """Transitively-minimal semaphore waits (see v2 docstring)."""
    UPD = ("sem-inc", "sem-add-imm")
    insts = [i for bb in nc.m.functions[0].blocks for i in bb.instructions]

    bad_sems = set()
    for i in insts:
        si = getattr(i, "sync_info", None)
        if si is None:
            continue
        for u in si.on_update:
            if u.sync_type != "semaphore" or u.update_mode not in UPD:
                bad_sems.add(u.id)

    def fifo_of(i):
        si = i.sync_info
        eng = str(getattr(i, "engine", None))
        if type(i).__name__ == "InstDMACopy" and si is not None:
            for u in si.on_update:
                if u.sync_type == "semaphore" and u.update_mode in UPD:
                    return ("q", u.id)
        return ("e", eng)

    cum: dict = {}
    event: dict = {}
    fifo_pred: dict = {}
    last_in_fifo: dict = {}
    metas = []
    for idx, i in enumerate(insts):
        si = getattr(i, "sync_info", None)
        f = fifo_of(i)
        fifo_pred[idx] = last_in_fifo.get(f)
        last_in_fifo[f] = idx
        ups = []
        if si is not None:
            for u in si.on_update:
                if u.sync_type == "semaphore" and u.update_mode in UPD:
                    cum[u.id] = cum.get(u.id, 0) + u.update_value
                    event[(u.id, cum[u.id])] = idx
                    ups.append((u.id, cum[u.id]))
        metas.append((si, ups))

    def resolve(sem, k):
        v = k
        while (sem, v) not in event:
            v += 1
            if v > cum.get(sem, 0):
                return None
        return event[(sem, v)]

    cvc: list = [None] * len(insts)

    def get_cvc(idx):
        if cvc[idx] is not None:
            return cvc[idx]
        stack = [idx]
        while stack:
            j = stack[-1]
            if cvc[j] is not None:
                stack.pop()
                continue
            si, ups = metas[j]
            deps = []
            p = fifo_pred[j]
            if p is not None:
                deps.append(p)
            if si is not None:
                for w in si.on_wait:
                    if (
                        w.sync_type == "semaphore"
                        and w.wait_mode == "sem-ge-imm"
                        and w.id not in bad_sems
                    ):
                        e = resolve(w.id, w.wait_value)
                        if e is not None and e != j:
                            deps.append(e)
            pending = [d for d in deps if cvc[d] is None]
            if pending:
                stack.extend(pending)
                continue
            stack.pop()
            vc: dict = {}
            for d in deps:
                for s, v in cvc[d].items():
                    if vc.get(s, 0) < v:
                        vc[s] = v
            if si is not None:
                for w in si.on_wait:
                    if (
                        w.sync_type == "semaphore"
                        and w.wait_mode == "sem-ge-imm"
                        and w.id not in bad_sems
                    ):
                        if vc.get(w.id, 0) < w.wait_value:
                            vc[w.id] = w.wait_value
            for s, v in ups:
                if vc.get(s, 0) < v:
                    vc[s] = v
            cvc[j] = vc
        return cvc[idx]

    for idx, i in enumerate(insts):
        si, _ups = metas[idx]
        if si is None or not si.on_wait:
            continue
        base: dict = {}
        p = fifo_pred[idx]
        if p is not None:
            base = dict(get_cvc(p))
        sem_waits = [
            w
            for w in si.on_wait
            if w.sync_type == "semaphore"
            and w.wait_mode == "sem-ge-imm"
            and w.id not in bad_sems
        ]
        other = [w for w in si.on_wait if w not in sem_waits]

        def strength(w):
            e = resolve(w.id, w.wait_value)
            return len(get_cvc(e)) if e is not None else 0

        sem_waits.sort(key=strength, reverse=True)

        def wait_cvc(w):
            e = resolve(w.id, w.wait_value)
            vc = dict(get_cvc(e)) if e is not None else {}
            if vc.get(w.id, 0) < w.wait_value:
                vc[w.id] = w.wait_value
            return vc

        kept = sem_waits[:]
        changed = True
        while changed:
            changed = False
            for w in kept:
                cover = dict(base)
                for w2 in kept:
                    if w2 is w:
                        continue
                    for s, v in wait_cvc(w2).items():
                        if cover.get(s, 0) < v:
                            cover[s] = v
                if cover.get(w.id, 0) >= w.wait_value:
                    kept.remove(w)
                    changed = True
                    break
        if len(kept) + len(other) != len(si.on_wait):
            si.on_wait = other + kept


def _build_nc(strip: bool = True) -> bass.Bass:
    nc = bass.Bass()
    ftd = nc.dram_tensor("ftd", [128, NQ, IB, QL], BF16, kind="ExternalInput")
    gtd = nc.dram_tensor("gtd", [128, NQ, JB, QL], BF16, kind="ExternalInput")
    wt = nc.dram_tensor("wt", [128, NCHUNK, 128], BF16, kind="ExternalInput")
    bias = nc.dram_tensor("bias", [128, 1], F32, kind="ExternalInput")
    out = nc.dram_tensor("out", [128, BD], BF16, kind="ExternalOutput")

    with tile.TileContext(nc) as tc:
        with (
            tc.tile_pool(name="res", bufs=1) as res,
            tc.tile_pool(name="pp", bufs=4) as ppool,
            tc.tile_pool(name="osb", bufs=4) as opool,
            tc.tile_pool(name="acc", bufs=8, space="PSUM") as apool,
        ):
            ftd_sb = res.tile([128, NQ * IB, QL], BF16)
            gtd_sb = res.tile([128, NQ * JB, QL], BF16)
            wt_sb = res.tile([128, NCHUNK, 128], BF16)
            bias_sb = res.tile([128, 1], F32)
            wl_sb = res.tile([128, 128], BF16, name="wl")
            wr_sb = res.tile([128, 512], BF16, name="wr")

            # PE warm-up: garbage matmuls on scratch tiles into the first
            # acc rotation slot keep the HAM clock busy during the DMA
            # ramp; bank is re-claimed by the last real accumulator long
            # after.  memset so CoreSim never sees uninitialized reads.
            nc.vector.memset(wl_sb[:], 0.0)
            nc.vector.memset(wr_sb[:], 0.0)
            warm = apool.tile([128, 512], F32, tag="acc")
            for k in range(NWARM):
                nc.tensor.matmul(warm[:], wl_sb[:], wr_sb[:],
                                 start=(k == 0), stop=(k == NWARM - 1))

            # streamed loads, h-major; first-use slices first.  NO touches
            # here -- touches happen on the consumer engine right before
            # first use so the first mul doesn't transitively wait on
            # every later load.
            nc.sync.dma_start(out=bias_sb[:], in_=bias[:])
            nc.sync.dma_start(out=gtd_sb[:, 0:JB, :], in_=gtd[:, 0, :, :])
            nc.sync.dma_start(out=ftd_sb[:, 0, :], in_=ftd[:, 0, 0, :])
            nc.sync.dma_start(out=ftd_sb[:, 1, :], in_=ftd[:, 0, 1, :])
            nc.sync.dma_start(out=wt_sb[:], in_=wt[:])
            for ib in range(2, IB):
                nc.sync.dma_start(out=ftd_sb[:, ib, :], in_=ftd[:, 0, ib, :])
            for h in range(1, NQ):
                nc.sync.dma_start(out=gtd_sb[:, h * JB:(h + 1) * JB, :],
                                  in_=gtd[:, h, :, :])
                nc.sync.dma_start(out=ftd_sb[:, h * IB:(h + 1) * IB, :],
                                  in_=ftd[:, h, :, :])

            for h in range(NQ):
                acc0 = apool.tile([128, 512], F32, tag="acc")
                acc1 = apool.tile([128, 512], F32, tag="acc")
                gsl = gtd_sb[:, h * JB:(h + 1) * JB, :]
                nc.vector.tensor_copy(gtd_sb[0:1, h * JB, 0:1],
                                      gtd_sb[0:1, h * JB, 0:1])
                if h > 0:
                    nc.vector.tensor_copy(ftd_sb[0:1, h * IB, 0:1],
                                          ftd_sb[0:1, h * IB, 0:1])
                for ib in range(IB):
                    if h == 0:
                        nc.vector.tensor_copy(ftd_sb[0:1, ib, 0:1],
                                              ftd_sb[0:1, ib, 0:1])
                    p2 = ppool.tile([128, JB, QL], BF16, tag="p")
                    fap = (ftd_sb[:, h * IB + ib, :][:, None, :]
                           .to_broadcast((128, JB, QL)))
                    nc.vector.tensor_mul(p2[:], gsl, fap)
                    for jb in range(JB):
                        c = JB * ib + jb
                        st = (ib == 0 and jb == 0)
                        sp = (ib == IB - 1 and jb == JB - 1)
                        nc.tensor.matmul(acc0[:], wt_sb[:, c, :],
                                         p2[:, jb, 0:512], start=st, stop=sp)
                        nc.tensor.matmul(acc1[:], wt_sb[:, c, :],
                                         p2[:, jb, 512:QL], start=st, stop=sp)
                for q, acc in ((0, acc0), (1, acc1)):
                    osb = opool.tile([128, 512], BF16, tag="osb")
                    nc.scalar.activation(osb[:], acc[:],
                                         mybir.ActivationFunctionType.Identity,
                                         bias=bias_sb[:, 0:1])
                    nc.sync.dma_start(
                        out=out[:, h * QL + q * 512: h * QL + (q + 1) * 512],
                        in_=osb[:])
                    nc.vector.tensor_copy(osb[0:1, 0:1], osb[0:1, 0:1])
    if strip:
        _strip_self_waits(nc)
    return nc


def _get_nc() -> bass.Bass:
    if "nc" not in _CACHE:
        _CACHE["nc"] = _build_nc()
    return _CACHE["nc"]


def _prep_core_inputs(feature, base, W, b, ci):
    bsl = slice(ci * BLOC, (ci + 1) * BLOC)
    F = np.asarray(feature[bsl], np.float32)  # (128, 64, 32)
    G = np.asarray(base[bsl], np.float32)     # (128, 64, 32)

    fT = np.transpose(F, (1, 0, 2)).reshape(HK, BD)   # (i, bd)
    gT = np.transpose(G, (1, 0, 2)).reshape(H0, BD)   # (j, bd)

    # ftd[p=(16a+b), h, ib, l] = fT[8*ib + a, h*QL + l]   (dup x16 over b)
    t = fT.reshape(IB, 8, NQ, QL).transpose(1, 2, 0, 3)      # (a, h, ib, l)
    ftd = np.broadcast_to(t[:, None], (8, 16, NQ, IB, QL))
    ftd = ftd.reshape(128, NQ, IB, QL)

    # gtd[p=(16a+b), h, jb, l] = gT[16*jb + b, h*QL + l]   (dup x8 over a)
    t = gT.reshape(JB, 16, NQ, QL).transpose(1, 2, 0, 3)     # (b, h, jb, l)
    gtd = np.broadcast_to(t[None], (8, 16, NQ, JB, QL))
    gtd = gtd.reshape(128, NQ, JB, QL)

    # wt[p=(16a+b), c=(4*ib+jb), o] = W[o, (8*ib+a)*64 + 16*jb + b]
    wt = np.asarray(W, np.float32).reshape(O, IB, 8, JB, 16)
    wt = wt.transpose(2, 4, 1, 3, 0).reshape(128, NCHUNK, O)

    return {
        "ftd": np.ascontiguousarray(ftd).astype(BF),
        "gtd": np.ascontiguousarray(gtd).astype(BF),
        "wt": np.ascontiguousarray(wt).astype(BF),
        "bias": np.ascontiguousarray(b, np.float32).reshape(128, 1),
    }


def run(feature, base, W, b, **spmd_kwargs):
    nc = _get_nc()
    in_maps = [_prep_core_inputs(feature, base, W, b, ci) for ci in range(NCORES)]
    res = run_bass_kernel_spmd(nc, in_maps, list(range(NCORES)), **spmd_kwargs)
    outs = []
    for ci in range(NCORES):
        o = np.asarray(res.results[ci]["out"], dtype=np.float32)
        o = o.reshape(O, BLOC, D)
        outs.append(np.transpose(o, (1, 0, 2)))
    full = np.concatenate(outs, 0)
    return full, res


def kernel(feature, base, W, b):
    full, _ = run(feature, base, W, b)
    return full
